# revision 1
# baseline (speedup 1.0000x reference)
"""Trainium2 Bass kernel for the KalmanFilterEstimator problem.

Math
----
Reference scan (per step, carry (x, P, L)):
    x_pred = x @ Wfx + bfx + u @ Wfu + bfu + d @ Wfd + bfd
    y      = x_pred @ Wfy + bfy
    P_pred = Wfx @ (P @ Wfx^T) + Q
    x_new  = x_pred + (ym - y) @ L^T            # L from the carry (previous step)
    S_inv  = inv(R + Wfy^T @ (P_pred @ Wfy))
    L_new  = (P_pred @ Wfy) @ S_inv
    P_new  = I - L_new @ (Wfy^T @ P_pred)
Only the final x is returned.

P/L are batch-independent, so the gain sequence L_t is precomputed on host
(float64 Riccati recursion). The x recurrence is then linear:
    x_{t+1} = x_t @ G_t + h_t,
    G_t = Wfx @ M_t,  M_t = I - Wfy @ L_t^T,
    h_t = (u_t@Wfu + d_t@Wfd + b) @ M_t + (ym_t - bfy) @ L_t^T,  b = bfx+bfu+bfd.
With x_0 = 0 and suffix products S_t = G_{t+1} ... G_{T-1}:
    x_T = sum_t [ ym_t @ (L_t^T S_t) + u_t @ (Wfu M_t S_t) + d_t @ (Wfd M_t S_t) ] + c
i.e. one tall-skinny matmul  x_T^T = WB^T @ ZT  with contraction over (t, feature).

The closed loop is strongly stable (||S_t|| decays ~0.3x per step for this
problem), so ||S_t|| underflows to exact f32 zero a few dozen steps from the
end; steps with ||S_t||_2 < 1e-10 contribute < 1e-9 relative and are skipped.
The cutoff is computed from the actual weights at runtime (keep >= 128 steps,
falls back to the full sequence if the loop were ever slow to forget).

Device kernel (per core): acc(64, 256) += wb_chunk(128, 64)^T @ zt_chunk(128, 256)
accumulated in PSUM over the core's contraction rows; cores split the
contraction dim (time); host sums the 8 partials and adds the constant c.
"""
import numpy as np

NCORES = 8
PART = 128  # SBUF partitions / matmul contraction tile
USE_F32R = False  # single-pass PE fp32 mode (validated per-problem before enabling)


def _precompute(Wfx, bfx, Wfu, bfu, Wfd, bfd, Wfy, bfy, T):
    f8 = np.float64
    Wfx = Wfx.astype(f8); Wfy = Wfy.astype(f8)
    Wfu = Wfu.astype(f8); Wfd = Wfd.astype(f8)
    b = bfx.astype(f8) + bfu.astype(f8) + bfd.astype(f8)
    bfy = bfy.astype(f8)
    nx = Wfx.shape[0]; ny = Wfy.shape[1]
    nu = Wfu.shape[0]; nd = Wfd.shape[0]
    I = np.eye(nx, dtype=f8)
    Q = np.eye(nx, dtype=f8)
    R = np.eye(ny, dtype=f8)

    Ls = np.empty((T, nx, ny), dtype=f8)
    P = np.eye(nx, dtype=f8)
    L = np.zeros((nx, ny), dtype=f8)
    for t in range(T):
        Ls[t] = L
        P = Wfx @ (P @ Wfx.T) + Q
        S_inv = np.linalg.inv(R + Wfy.T @ (P @ Wfy))
        L_new = (P @ Wfy) @ S_inv
        P = I - L_new @ (Wfy.T @ P)
        L = L_new

    Ay = np.empty((T, ny, nx), dtype=f8)
    Au = np.empty((T, nu, nx), dtype=f8)
    Ad = np.empty((T, nd, nx), dtype=f8)
    snorm = np.empty(T, dtype=f8)
    c = np.zeros(nx, dtype=f8)
    S = np.eye(nx, dtype=f8)
    for t in range(T - 1, -1, -1):
        M = I - Wfy @ Ls[t].T
        MS = M @ S
        LTS = Ls[t].T @ S
        Ay[t] = LTS
        Au[t] = Wfu @ MS
        Ad[t] = Wfd @ MS
        c += b @ MS - bfy @ LTS
        snorm[t] = np.linalg.norm(S, 2)
        S = (Wfx @ M) @ S
    return Ay, Au, Ad, c, snorm


def _build_bass(kc, nb, nx, use_f32r=False):
    """Per-core program: acc(nx, nb) = sum over 128-row chunks of
    zw[:, nb:nb+nx]^T @ zw[:, 0:nb], where zw (kc, nb+nx) packs the moving
    (zt) and stationary (wb) operands side by side so each chunk group
    arrives in ONE DMA.

    Raw Bass (no TileContext): this walrus build rejects instructions with
    more than ~one sync wait ("Too many sync wait commands"), which Tile's
    closing Drain (4 waits) trips. With explicit semaphores every
    instruction carries at most one wait.
    """
    import concourse.bass as bass
    import concourse.mybir as mybir

    f32 = mybir.dt.float32
    # float32r: same fp32 bytes, single-pass PE matmul (1 cyc/row at N>=256)
    # instead of the two-pass LOW/HIGH fp32 decomposition
    mmdt = mybir.dt.float32r if use_f32r else f32
    nf2 = nb + nx
    nc = bass.Bass(enable_partition_id=False, monotonic_sem_count=0)
    zw = nc.dram_tensor("zw", [kc, nf2], mmdt, kind="ExternalInput")
    acc = nc.dram_tensor("acc", [nx, nb], f32, kind="ExternalOutput")
    nchunks = kc // PART
    NSLOT = min(nchunks, 8)  # in-flight chunk slots (each its own DMA queue)

    with (
        nc.sbuf_tensor([PART, NSLOT, nf2], mmdt) as zwt,
        nc.sbuf_tensor([nx, nb], f32) as outt,
        nc.psum_tensor([nx, nb], f32) as ps,
        nc.Block() as block,
        # per-slot DMA-completion sems so each matmul carries exactly one wait
        _multisem(nc, NSLOT) as dsems,
        nc.semaphore() as psem,    # matmuls retired (for slot reuse)
        nc.semaphore() as vsem,    # PSUM->SBUF copy done
        nc.semaphore() as osem,    # output DMA done
    ):
        @block.sync
        def _(sync):
            for i in range(nchunks):
                s = i % NSLOT
                if i >= NSLOT:
                    # WAR: slot s reused; its previous chunk's matmul retired?
                    sync.wait_ge(psem, i - NSLOT + 1)
                sync.dma_start(
                    zwt[:, s, :], zw[i * PART:(i + 1) * PART, :]
                ).then_inc(dsems[s], 16)
            sync.wait_ge(vsem, 1)
            sync.dma_start(acc[:, :], outt[:]).then_inc(osem, 16)
            sync.wait_ge(osem, 16)  # keep SP alive until the result landed

        @block.tensor
        def _(tensor):
            for i in range(nchunks):
                s = i % NSLOT
                tensor.wait_ge(dsems[s], 16 * (i // NSLOT + 1))
                nc.tensor.matmul(
                    ps[:], zwt[:, s, nb:nf2], zwt[:, s, 0:nb],
                    start=(i == 0), stop=(i == nchunks - 1),
                ).then_inc(psem, 1)

        @block.vector
        def _(vector):
            vector.wait_ge(psem, nchunks)
            nc.vector.tensor_copy(outt[:], ps[:]).then_inc(vsem, 1)

    return nc


def _multisem(nc, n):
    from contextlib import ExitStack, contextmanager

    @contextmanager
    def _cm():
        with ExitStack() as es:
            yield [es.enter_context(nc.semaphore(f"dsem{i}")) for i in range(n)]
    return _cm()


def _prepare(inputs):
    """Host precompute + data marshalling. Returns (in_maps, nc, cvec, meta)."""
    Ym = np.asarray(inputs["Ym"]); U = np.asarray(inputs["U"]); D = np.asarray(inputs["D"])
    T, B, ny = Ym.shape
    nu = U.shape[2]; nd = D.shape[2]
    nx = np.asarray(inputs["Wfx"]).shape[0]
    nf = ny + nu + nd

    Ay, Au, Ad, cvec, snorm = _precompute(
        np.asarray(inputs["Wfx"]), np.asarray(inputs["bfx"]),
        np.asarray(inputs["Wfu"]), np.asarray(inputs["bfu"]),
        np.asarray(inputs["Wfd"]), np.asarray(inputs["bfd"]),
        np.asarray(inputs["Wfy"]), np.asarray(inputs["bfy"]), T)

    # steps with ||S_t|| < 1e-10 contribute < ~1e-9 relative; keep a 64-step
    # margin and round so each core's row count is a multiple of 128
    cut = int(np.argmax(snorm > 1e-10))
    keep = T - cut + 64
    step_quantum = (NCORES * PART) // np.gcd(NCORES * PART, nf)
    keep = min(T, -(-keep // step_quantum) * step_quantum)
    s = T - keep

    Z = np.concatenate([Ym[s:], U[s:], D[s:]], axis=2)          # (keep, B, nf)
    ZT = np.ascontiguousarray(Z.transpose(0, 2, 1)).reshape(keep * nf, B)
    ZT = ZT.astype(np.float32, copy=False)
    WB = np.concatenate([Ay[s:], Au[s:], Ad[s:]], axis=1).reshape(keep * nf, nx)
    WB = WB.astype(np.float32)
    # pack moving + stationary operands side by side: (K, B+nx)
    ZW = np.concatenate([ZT, WB], axis=1)

    kc = (keep * nf) // NCORES
    assert kc % PART == 0, (keep, nf, kc)
    in_maps = [
        {"zw": np.ascontiguousarray(ZW[c * kc:(c + 1) * kc])}
        for c in range(NCORES)
    ]
    nc = _build_bass(kc, B, nx, use_f32r=USE_F32R)
    return in_maps, nc, cvec, dict(keep=keep, kc=kc, B=B, nx=nx, f32r=USE_F32R)


def _finish(results, cvec):
    accT = np.zeros_like(results[0]["acc"], dtype=np.float64)
    for r in results:
        accT += r["acc"]
    return (accT.T + cvec).astype(np.float32)


def kernel(**inputs):
    from concourse.bass_utils import run_bass_kernel_spmd
    in_maps, nc, cvec, _ = _prepare(inputs)
    res = run_bass_kernel_spmd(nc, in_maps, core_ids=list(range(NCORES)))
    return _finish(res.results, cvec)



# revision 3
# speedup vs baseline: 1.1941x; 1.1941x over previous
"""Trainium2 Bass kernel for the KalmanFilterEstimator problem.

Math
----
Reference scan (per step, carry (x, P, L)):
    x_pred = x @ Wfx + bfx + u @ Wfu + bfu + d @ Wfd + bfd
    y      = x_pred @ Wfy + bfy
    P_pred = Wfx @ (P @ Wfx^T) + Q
    x_new  = x_pred + (ym - y) @ L^T            # L from the carry (previous step)
    S_inv  = inv(R + Wfy^T @ (P_pred @ Wfy))
    L_new  = (P_pred @ Wfy) @ S_inv
    P_new  = I - L_new @ (Wfy^T @ P_pred)
Only the final x is returned.

P/L are batch-independent, so the gain sequence L_t is precomputed on host
(float64 Riccati recursion). The x recurrence is then linear:
    x_{t+1} = x_t @ G_t + h_t,
    G_t = Wfx @ M_t,  M_t = I - Wfy @ L_t^T,
    h_t = (u_t@Wfu + d_t@Wfd + b) @ M_t + (ym_t - bfy) @ L_t^T,  b = bfx+bfu+bfd.
With x_0 = 0 and suffix products S_t = G_{t+1} ... G_{T-1}:
    x_T = sum_t [ ym_t @ (L_t^T S_t) + u_t @ (Wfu M_t S_t) + d_t @ (Wfd M_t S_t) ] + c
i.e. one tall-skinny matmul  x_T^T = WB^T @ ZT  with contraction over (t, feature).

The closed loop is strongly stable (||S_t|| ~0.42x per step here), so only the
last ~23 steps contribute above 1e-7 relative; the cutoff is computed from the
actual weights at runtime via the tail-sum bound sum_{dropped t} ||S_t|| < 1e-7
(falls back to more chunks if the loop were ever slow to forget).

Device kernel (per core): acc(64, 256) = sum over the core's 128-row chunks of
zw[:, 256:320]^T @ zw[:, 0:256], accumulated in PSUM; zw packs the moving (ZT)
and stationary (WB) operands side by side so each chunk arrives in ONE DMA.
For this problem the kept tail is 1024 rows total -> exactly one chunk per core:
one input DMA, one matmul group, one PSUM->SBUF copy, one output DMA.
Host pads with zero rows, sums the 8 partials and adds the constant c.
"""
import numpy as np

NCORES = 8
PART = 128  # SBUF partitions / matmul contraction tile
DTYPE = "f32"  # "f32" | "f32r" | "bf16"  (device matmul dtype)
TAIL_TOL = 1e-7  # tail-sum bound on dropped ||S_t|| mass


def _precompute(Wfx, bfx, Wfu, bfu, Wfd, bfd, Wfy, bfy, T):
    f8 = np.float64
    Wfx = Wfx.astype(f8); Wfy = Wfy.astype(f8)
    Wfu = Wfu.astype(f8); Wfd = Wfd.astype(f8)
    b = bfx.astype(f8) + bfu.astype(f8) + bfd.astype(f8)
    bfy = bfy.astype(f8)
    nx = Wfx.shape[0]; ny = Wfy.shape[1]
    nu = Wfu.shape[0]; nd = Wfd.shape[0]
    I = np.eye(nx, dtype=f8)
    Q = np.eye(nx, dtype=f8)
    R = np.eye(ny, dtype=f8)

    Ls = np.empty((T, nx, ny), dtype=f8)
    P = np.eye(nx, dtype=f8)
    L = np.zeros((nx, ny), dtype=f8)
    for t in range(T):
        Ls[t] = L
        P = Wfx @ (P @ Wfx.T) + Q
        S_inv = np.linalg.inv(R + Wfy.T @ (P @ Wfy))
        L_new = (P @ Wfy) @ S_inv
        P = I - L_new @ (Wfy.T @ P)
        L = L_new

    Ay = np.empty((T, ny, nx), dtype=f8)
    Au = np.empty((T, nu, nx), dtype=f8)
    Ad = np.empty((T, nd, nx), dtype=f8)
    snorm = np.empty(T, dtype=f8)
    c = np.zeros(nx, dtype=f8)
    S = np.eye(nx, dtype=f8)
    for t in range(T - 1, -1, -1):
        M = I - Wfy @ Ls[t].T
        MS = M @ S
        LTS = Ls[t].T @ S
        Ay[t] = LTS
        Au[t] = Wfu @ MS
        Ad[t] = Wfd @ MS
        c += b @ MS - bfy @ LTS
        snorm[t] = np.linalg.norm(S, 2)
        S = (Wfx @ M) @ S
    return Ay, Au, Ad, c, snorm


def _mybir_dtype(name):
    import concourse.mybir as mybir
    return {"f32": mybir.dt.float32, "f32r": mybir.dt.float32r,
            "bf16": mybir.dt.bfloat16}[name]


def _np_dtype(name):
    if name == "bf16":
        import ml_dtypes
        return ml_dtypes.bfloat16
    return np.float32


def _multisem(nc, n):
    from contextlib import ExitStack, contextmanager

    @contextmanager
    def _cm():
        with ExitStack() as es:
            yield [es.enter_context(nc.semaphore(f"dsem{i}")) for i in range(n)]
    return _cm()


def _build(kc, nb, nx, dtype):
    import concourse.bass as bass
    import concourse.mybir as mybir

    f32 = mybir.dt.float32
    mmdt = _mybir_dtype(dtype)
    nf2 = nb + nx
    nc = bass.Bass(enable_partition_id=False, monotonic_sem_count=0)
    zw = nc.dram_tensor("zw", [kc, nf2], mmdt, kind="ExternalInput")
    acc = nc.dram_tensor("acc", [nx, nb], f32, kind="ExternalOutput")
    nchunks = kc // PART

    with (
        nc.sbuf_tensor([PART, nchunks, nf2], mmdt) as zwt,
        nc.sbuf_tensor([nx, nb], f32) as outt,
        nc.psum_tensor([nx, nb], f32) as ps,
        nc.Block() as block,
        _multisem(nc, nchunks) as dsems,
        nc.semaphore() as psem,    # matmuls retired (vector waits)
        nc.semaphore() as vsem,    # PSUM->SBUF copy done
        nc.semaphore() as osem,    # output DMA done
    ):
        @block.sync
        def _(sync):
            for i in range(nchunks):
                sync.dma_start(
                    zwt[:, i, :], zw[i * PART:(i + 1) * PART, :]
                ).then_inc(dsems[i], 16)
            sync.wait_ge(vsem, 1)
            sync.dma_start(acc[:, :], outt[:]).then_inc(osem, 16)
            sync.wait_ge(osem, 16)  # keep SP alive until the result landed

        @block.tensor
        def _(tensor):
            for i in range(nchunks):
                tensor.wait_ge(dsems[i], 16)
                nc.tensor.matmul(
                    ps[:], zwt[:, i, nb:nf2], zwt[:, i, 0:nb],
                    start=(i == 0), stop=(i == nchunks - 1),
                ).then_inc(psem, 1)

        @block.vector
        def _(vector):
            vector.wait_ge(psem, nchunks)
            nc.vector.tensor_copy(outt[:], ps[:]).then_inc(vsem, 1)

    return nc


def _prepare(inputs, dtype=None):
    """Host precompute + data marshalling. Returns (in_maps, nc, cvec, meta)."""
    dtype = dtype or DTYPE
    Ym = np.asarray(inputs["Ym"]); U = np.asarray(inputs["U"]); D = np.asarray(inputs["D"])
    T, B, ny = Ym.shape
    nu = U.shape[2]; nd = D.shape[2]
    nx = np.asarray(inputs["Wfx"]).shape[0]
    nf = ny + nu + nd

    Ay, Au, Ad, cvec, snorm = _precompute(
        np.asarray(inputs["Wfx"]), np.asarray(inputs["bfx"]),
        np.asarray(inputs["Wfu"]), np.asarray(inputs["bfu"]),
        np.asarray(inputs["Wfd"]), np.asarray(inputs["bfd"]),
        np.asarray(inputs["Wfy"]), np.asarray(inputs["bfy"]), T)

    # smallest keep whose dropped tail mass sum ||S_t|| stays under TAIL_TOL
    tailsum = np.cumsum(snorm)  # tailsum[t] = sum of snorm[0..t]
    drop = np.searchsorted(tailsum, TAIL_TOL)  # max t with sum <= tol
    keep = T - max(0, drop - 1)
    keep = min(T, max(keep, 8))
    s = T - keep

    rows = keep * nf
    # pad rows with zeros so each core gets an equal multiple of 128
    kc = PART * (-(-rows // (PART * NCORES)))
    rows_pad = kc * NCORES

    Z = np.concatenate([Ym[s:], U[s:], D[s:]], axis=2)          # (keep, B, nf)
    ZT = np.ascontiguousarray(Z.transpose(0, 2, 1)).reshape(rows, B)
    WB = np.concatenate([Ay[s:], Au[s:], Ad[s:]], axis=1).reshape(rows, nx)
    npdt = _np_dtype(dtype)
    ZW = np.zeros((rows_pad, B + nx), dtype=npdt)
    ZW[:rows, :B] = ZT.astype(npdt)
    ZW[:rows, B:] = WB.astype(npdt)

    in_maps = [
        {"zw": np.ascontiguousarray(ZW[c * kc:(c + 1) * kc])}
        for c in range(NCORES)
    ]
    nc = _build(kc, B, nx, dtype)
    return in_maps, nc, cvec, dict(keep=keep, kc=kc, B=B, nx=nx, dtype=dtype)


def _finish(results, cvec):
    accT = np.zeros_like(results[0]["acc"], dtype=np.float64)
    for r in results:
        accT += r["acc"]
    return (accT.T + cvec).astype(np.float32)


def kernel(**inputs):
    from concourse.bass_utils import run_bass_kernel_spmd
    in_maps, nc, cvec, _ = _prepare(inputs)
    res = run_bass_kernel_spmd(nc, in_maps, core_ids=list(range(NCORES)))
    return _finish(res.results, cvec)


# revision 5
# speedup vs baseline: 1.4030x; 1.1749x over previous
"""Trainium2 Bass kernel for the KalmanFilterEstimator problem.

Math
----
Reference scan (per step, carry (x, P, L)):
    x_pred = x @ Wfx + bfx + u @ Wfu + bfu + d @ Wfd + bfd
    y      = x_pred @ Wfy + bfy
    P_pred = Wfx @ (P @ Wfx^T) + Q
    x_new  = x_pred + (ym - y) @ L^T            # L from the carry (previous step)
    S_inv  = inv(R + Wfy^T @ (P_pred @ Wfy))
    L_new  = (P_pred @ Wfy) @ S_inv
    P_new  = I - L_new @ (Wfy^T @ P_pred)
Only the final x is returned.

P/L are batch-independent, so the gain sequence L_t is precomputed on host
(float64 Riccati recursion). The x recurrence is then linear:
    x_{t+1} = x_t @ G_t + h_t,
    G_t = Wfx @ M_t,  M_t = I - Wfy @ L_t^T,
    h_t = (u_t@Wfu + d_t@Wfd + b) @ M_t + (ym_t - bfy) @ L_t^T,  b = bfx+bfu+bfd.
With x_0 = 0 and suffix products S_t = G_{t+1} ... G_{T-1}:
    x_T = sum_t [ ym_t @ (L_t^T S_t) + u_t @ (Wfu M_t S_t) + d_t @ (Wfd M_t S_t) ] + c
i.e. one tall-skinny matmul  x_T^T = WB^T @ ZT  with contraction over (t, feature).

The closed loop is strongly stable (||S_t|| ~0.42x per step here), so only the
last ~23 steps contribute above 1e-7 relative; the cutoff is computed from the
actual weights at runtime via the tail-sum bound sum_{dropped t} ||S_t|| < 1e-7
(falls back to more chunks if the loop were ever slow to forget).

Device kernel (per core): acc(64, 256) = sum over the core's 128-row chunks of
zw[:, 256:320]^T @ zw[:, 0:256], accumulated in PSUM; zw packs the moving (ZT)
and stationary (WB) operands side by side so each chunk arrives in ONE DMA.
For this problem the kept tail is 1024 rows total -> exactly one chunk per core:
one input DMA, one matmul group, one PSUM->SBUF copy, one output DMA.
Host pads with zero rows, sums the 8 partials and adds the constant c.
"""
import numpy as np

import os

NCORES = 8
PART = 128  # SBUF partitions / matmul contraction tile
DTYPE = os.environ.get("KF_DTYPE", "f32")  # "f32" | "f32r" | "bf16"
WAIT_OUT = os.environ.get("KF_WAIT_OUT", "1") == "1"  # final osem wait
TAIL_TOL = 1e-7  # tail-sum bound on dropped ||S_t|| mass


def _precompute(Wfx, bfx, Wfu, bfu, Wfd, bfd, Wfy, bfy, T):
    f8 = np.float64
    Wfx = Wfx.astype(f8); Wfy = Wfy.astype(f8)
    Wfu = Wfu.astype(f8); Wfd = Wfd.astype(f8)
    b = bfx.astype(f8) + bfu.astype(f8) + bfd.astype(f8)
    bfy = bfy.astype(f8)
    nx = Wfx.shape[0]; ny = Wfy.shape[1]
    nu = Wfu.shape[0]; nd = Wfd.shape[0]
    I = np.eye(nx, dtype=f8)
    Q = np.eye(nx, dtype=f8)
    R = np.eye(ny, dtype=f8)

    Ls = np.empty((T, nx, ny), dtype=f8)
    P = np.eye(nx, dtype=f8)
    L = np.zeros((nx, ny), dtype=f8)
    for t in range(T):
        Ls[t] = L
        P = Wfx @ (P @ Wfx.T) + Q
        S_inv = np.linalg.inv(R + Wfy.T @ (P @ Wfy))
        L_new = (P @ Wfy) @ S_inv
        P = I - L_new @ (Wfy.T @ P)
        L = L_new

    Ay = np.empty((T, ny, nx), dtype=f8)
    Au = np.empty((T, nu, nx), dtype=f8)
    Ad = np.empty((T, nd, nx), dtype=f8)
    snorm = np.empty(T, dtype=f8)
    c = np.zeros(nx, dtype=f8)
    S = np.eye(nx, dtype=f8)
    for t in range(T - 1, -1, -1):
        M = I - Wfy @ Ls[t].T
        MS = M @ S
        LTS = Ls[t].T @ S
        Ay[t] = LTS
        Au[t] = Wfu @ MS
        Ad[t] = Wfd @ MS
        c += b @ MS - bfy @ LTS
        snorm[t] = np.linalg.norm(S, 2)
        S = (Wfx @ M) @ S
    return Ay, Au, Ad, c, snorm


def _mybir_dtype(name):
    import concourse.mybir as mybir
    return {"f32": mybir.dt.float32, "f32r": mybir.dt.float32r,
            "bf16": mybir.dt.bfloat16}[name]


def _np_dtype(name):
    if name == "bf16":
        import ml_dtypes
        return ml_dtypes.bfloat16
    return np.float32


def _multisem(nc, n):
    from contextlib import ExitStack, contextmanager

    @contextmanager
    def _cm():
        with ExitStack() as es:
            yield [es.enter_context(nc.semaphore(f"dsem{i}")) for i in range(n)]
    return _cm()


def _build(kc, nb, nx, dtype):
    import concourse.bass as bass
    import concourse.mybir as mybir

    f32 = mybir.dt.float32
    mmdt = _mybir_dtype(dtype)
    nf2 = nb + nx
    nc = bass.Bass(enable_partition_id=False, monotonic_sem_count=0)
    zw = nc.dram_tensor("zw", [kc, nf2], mmdt, kind="ExternalInput")
    acc = nc.dram_tensor("acc", [nx, nb], f32, kind="ExternalOutput")
    nchunks = kc // PART

    with (
        nc.sbuf_tensor([PART, nchunks, nf2], mmdt) as zwt,
        nc.sbuf_tensor([nx, nb], f32) as outt,
        nc.psum_tensor([nx, nb], f32) as ps,
        nc.Block() as block,
        _multisem(nc, nchunks) as dsems,
        nc.semaphore() as psem,    # matmuls retired (vector waits)
        nc.semaphore() as vsem,    # PSUM->SBUF copy done
        nc.semaphore() as osem,    # output DMA done
    ):
        @block.sync
        def _(sync):
            for i in range(nchunks):
                sync.dma_start(
                    zwt[:, i, :], zw[i * PART:(i + 1) * PART, :]
                ).then_inc(dsems[i], 16)
            sync.wait_ge(vsem, 1)
            sync.dma_start(acc[:, :], outt[:]).then_inc(osem, 16)
            if WAIT_OUT:
                sync.wait_ge(osem, 16)  # keep SP alive until the result landed

        @block.tensor
        def _(tensor):
            for i in range(nchunks):
                tensor.wait_ge(dsems[i], 16)
                nc.tensor.matmul(
                    ps[:], zwt[:, i, nb:nf2], zwt[:, i, 0:nb],
                    start=(i == 0), stop=(i == nchunks - 1),
                ).then_inc(psem, 1)

        @block.vector
        def _(vector):
            vector.wait_ge(psem, nchunks)
            nc.vector.tensor_copy(outt[:], ps[:]).then_inc(vsem, 1)

    return nc


def _prepare(inputs, dtype=None):
    """Host precompute + data marshalling. Returns (in_maps, nc, cvec, meta)."""
    dtype = dtype or DTYPE
    Ym = np.asarray(inputs["Ym"]); U = np.asarray(inputs["U"]); D = np.asarray(inputs["D"])
    T, B, ny = Ym.shape
    nu = U.shape[2]; nd = D.shape[2]
    nx = np.asarray(inputs["Wfx"]).shape[0]
    nf = ny + nu + nd

    Ay, Au, Ad, cvec, snorm = _precompute(
        np.asarray(inputs["Wfx"]), np.asarray(inputs["bfx"]),
        np.asarray(inputs["Wfu"]), np.asarray(inputs["bfu"]),
        np.asarray(inputs["Wfd"]), np.asarray(inputs["bfd"]),
        np.asarray(inputs["Wfy"]), np.asarray(inputs["bfy"]), T)

    # smallest keep whose dropped tail mass sum ||S_t|| stays under TAIL_TOL
    tailsum = np.cumsum(snorm)  # tailsum[t] = sum of snorm[0..t]
    drop = np.searchsorted(tailsum, TAIL_TOL)  # max t with sum <= tol
    keep = T - max(0, drop - 1)
    keep = min(T, max(keep, 8))
    s = T - keep

    rows = keep * nf
    # pad rows with zeros so each core gets an equal multiple of 128
    kc = PART * (-(-rows // (PART * NCORES)))
    rows_pad = kc * NCORES

    Z = np.concatenate([Ym[s:], U[s:], D[s:]], axis=2)          # (keep, B, nf)
    ZT = np.ascontiguousarray(Z.transpose(0, 2, 1)).reshape(rows, B)
    WB = np.concatenate([Ay[s:], Au[s:], Ad[s:]], axis=1).reshape(rows, nx)
    npdt = _np_dtype(dtype)
    ZW = np.zeros((rows_pad, B + nx), dtype=npdt)
    ZW[:rows, :B] = ZT.astype(npdt)
    ZW[:rows, B:] = WB.astype(npdt)

    in_maps = [
        {"zw": np.ascontiguousarray(ZW[c * kc:(c + 1) * kc])}
        for c in range(NCORES)
    ]
    nc = _build(kc, B, nx, dtype)
    return in_maps, nc, cvec, dict(keep=keep, kc=kc, B=B, nx=nx, dtype=dtype)


def _finish(results, cvec):
    accT = np.zeros_like(results[0]["acc"], dtype=np.float64)
    for r in results:
        accT += r["acc"]
    return (accT.T + cvec).astype(np.float32)


def kernel(**inputs):
    from concourse.bass_utils import run_bass_kernel_spmd
    in_maps, nc, cvec, _ = _prepare(inputs)
    res = run_bass_kernel_spmd(nc, in_maps, core_ids=list(range(NCORES)))
    return _finish(res.results, cvec)
